# revision 31
# baseline (speedup 1.0000x reference)
"""CapsuleLayer (dynamic routing) Trainium2 kernel, v3: no materialized u_hat.

Data-parallel over batch across 8 NeuronCores (32 samples/core).

Per core and per routing pass, everything is derived from x and W directly:
  - agreement a[b,i,j] = sum_d x[b,i,d] * Wv[b,i,j,d], where
    Wv = sum_e W[i,j,d,e] v[b,j,e] comes from PE matmuls with a
    block-diagonal v moving tile (contraction over (j,e)); the
    d-reduction runs on PE with 0/1 selector stationaries;
  - s[b,j,e] = sum_i sum_d W[i,j,d,e] c[b,i,j] x[b,i,d] on PE,
    contracting over capsules in the compact (c,il) partition layout with
    one stationary per d -- so c (softmax output) is consumed straight
    from SBUF with no PSUM round-trip;
  - PSUM evacuations round-robin over ACT/Pool/DVE; x-broadcast muls are
    gang-batched fp16 tensor_tensor (DVE 2x mode, some chunks on Pool);
  - softmax on ACT/DVE; squash via ln/exp (one ACT table).
No u_hat tensor is ever stored: SBUF holds only W layouts, x layouts and
[b,i,j]-sized routing state.

Layouts (per core):
  "spread": partition p = il*8+d, groups g of 16 capsules (72 groups);
  "compact": partition q = c*16+il, gangs gg of 8 groups (9 gangs);
  (j,e) packed j-major: je = j*16+e; routing cols (j,b): j*32+b.

Self-contained: needs numpy + the concourse package on sys.path.
"""
import numpy as np
import concourse.bacc as bacc
import concourse.tile as tile
from concourse import mybir
from concourse.bass_utils import run_bass_kernel_spmd

B, Nin, Din, Nout, Dout = 256, 1152, 8, 10, 16
NCORES = 8
BL = B // NCORES      # 32 samples per core
G = Nin // 16         # 72 groups of 16 input capsules
GG = G // 8           # 9 gangs of 8 groups
JE = Nout * Dout      # 160, j-major: je = j*16 + e
JB = Nout * BL        # 320 routing cols: j*32 + b
EPS = 1e-7

F16 = mybir.dt.float16
F32 = mybir.dt.float32
AL = mybir.AluOpType
AF = mybir.ActivationFunctionType
AX = mybir.AxisListType

CFG = {
    # psum->sbuf f16 evacuation engine schedule (cycled per evac op):
    # 'a'=ACT, 'p'=Pool, 'v'=DVE
    "evac_rr": "a",
    # of each gang-batched x-mul (8 sub-chunks), how many go to Pool
    "mul_pool": 2,
    # ablation switches (timing experiments only; break correctness)
    "no_a": False, "no_evac": False, "no_mul": False,
    "no_c": False, "no_d": False, "no_soft": False,
    "order": "adc",   # per-cycle PE emission: a+consume(D)+softmax-c or a+c+D
    "lag": 2,         # gangs of lag between softmax and consume
    "xwbufs": 3,
}


# ----------------------------------------------------------------- host prep
def _prep_xd(xc):
    """xc [BL, Nin, Din] fp32 -> xd [128, G, BL] f16 (spread layout):
    xd[il*8+d, g, b] = xc[b, g*16+il, d]"""
    t = xc.reshape(BL, G, 16, 8).transpose(2, 3, 1, 0)  # [il,d,g,b]
    return np.ascontiguousarray(t.reshape(128, G, BL)).astype(np.float16)


def _prep_xq(xc):
    """xc [BL, Nin, Din] fp32 -> xq [128, GG, 8, BL] f16 (compact layout):
    xq[c*16+il, gg, d, b] = xc[b, (gg*8+c)*16+il, d]"""
    t = xc.reshape(BL, GG, 8, 16, 8).transpose(2, 3, 1, 4, 0)  # c,il,gg,d,b
    return np.ascontiguousarray(t.reshape(128, GG, 8, BL)).astype(np.float16)


def _prep_wq(W):
    """W -> wq [128, GG, 8, JE] f16 (compact layout):
    wq[c*16+il, gg, d, j*16+e] = W[(gg*8+c)*16+il, j, d, e]"""
    W6 = W.reshape(GG, 8, 16, Nout, Din, Dout)   # gg,c,il,j,d,e
    t = W6.transpose(1, 2, 0, 4, 3, 5)           # c,il,gg,d,j,e
    return np.ascontiguousarray(t.reshape(128, GG, 8, JE)).astype(np.float16)


def _prep_wg(W):
    """W -> wg [128, G, JE] f16: wg[il*8+d, g, j*16+e] = W[g*16+il, j, d, e]"""
    W6 = W.reshape(G, 16, Nout, Din, Dout)       # g,il,j,d,e
    t = W6.transpose(1, 3, 0, 2, 4)              # il,d,g,j,e
    return np.ascontiguousarray(t.reshape(128, G, JE)).astype(np.float16)


def _prep_wT(W):
    """W -> wT [JE, G, 128] f16: wT[j*16+e, g, il*8+d] = W[g*16+il, j, d, e];
    split into j0-7 [128, G, 128] and j8-9 [32, G, 128]."""
    W6 = W.reshape(G, 16, Nout, Din, Dout)
    t = W6.transpose(2, 4, 0, 1, 3)              # j,e,g,il,d
    t = np.ascontiguousarray(t.reshape(JE, G, 128)).astype(np.float16)
    return np.ascontiguousarray(t[:128]), np.ascontiguousarray(t[128:])


def _consts():
    il = np.arange(16)
    d = np.arange(8)
    sel = np.zeros((8, 128, 128), np.float16)    # Tsel: d-reduce selectors
    for c in range(8):
        for i in il:
            sel[c, i * 8 + d, c * 16 + i] = 1.0
    ident = np.eye(128, dtype=np.float32)
    jm7 = np.zeros((128, 8), np.float16)
    for j in range(8):
        jm7[16 * j:16 * j + 16, j] = 1.0
    jm8 = np.zeros((32, 2), np.float16)
    for jj in range(2):
        jm8[16 * jj:16 * jj + 16, jj] = 1.0
    return sel, ident, jm7, jm8


# -------------------------------------------------------------- device build
def _build_program(reps=1, inner=1):
    nc = bacc.Bacc("TRN2", target_bir_lowering=False, debug=False,
                   num_devices=NCORES)
    xd = nc.dram_tensor("xd", [128, G, BL], F16, kind="ExternalInput").ap()
    xq = nc.dram_tensor("xq", [128, GG, 8, BL], F16,
                        kind="ExternalInput").ap()
    wq = nc.dram_tensor("wq", [128, GG, 8, JE], F16,
                        kind="ExternalInput").ap()
    wg = nc.dram_tensor("wg", [128, G, JE], F16,
                        kind="ExternalInput").ap()
    wt07 = nc.dram_tensor("wt07", [128, G, 128], F16,
                          kind="ExternalInput").ap()
    wt89 = nc.dram_tensor("wt89", [32, G, 128], F16,
                          kind="ExternalInput").ap()
    sel = nc.dram_tensor("sel", [8, 128, 128], F16,
                         kind="ExternalInput").ap()
    ident = nc.dram_tensor("ident", [128, 128], F32,
                           kind="ExternalInput").ap()
    jm7 = nc.dram_tensor("jm7", [128, 8], F16, kind="ExternalInput").ap()
    jm8 = nc.dram_tensor("jm8", [32, 2], F16, kind="ExternalInput").ap()
    vout = nc.dram_tensor("vout", [BL, JE], F32,  # j-major
                          kind="ExternalOutput").ap()

    with tile.TileContext(nc) as tc:
        with (
            tc.tile_pool(name="const", bufs=1) as cpool,
            tc.tile_pool(name="state", bufs=1) as stpool,
            tc.tile_pool(name="xwv", bufs=CFG["xwbufs"]) as xwpool,
            tc.tile_pool(name="cx", bufs=CFG["xwbufs"]) as cxpool,
            tc.tile_pool(name="exp", bufs=3) as epool,
            tc.tile_pool(name="small", bufs=4) as spool,
            tc.tile_pool(name="bd", bufs=2) as bdpool,
            tc.tile_pool(name="pWv", bufs=2, space="PSUM") as pWv,
            tc.tile_pool(name="pA", bufs=2, space="PSUM") as pA,
            tc.tile_pool(name="pS", bufs=1, space="PSUM") as pS,
        ):
            xd_sb = cpool.tile([128, G, BL], F16)
            nc.sync.dma_start(out=xd_sb[:], in_=xd[:])
            xq_sb = cpool.tile([128, GG, 8, BL], F16)
            nc.sync.dma_start(out=xq_sb[:], in_=xq[:])
            wq_sb = cpool.tile([128, GG, 8, JE], F16)
            for w0 in range(0, GG, 3):
                nc.sync.dma_start(out=wq_sb[:, w0:w0 + 3, :, :],
                                  in_=wq[:, w0:w0 + 3, :, :])
            wg_sb = cpool.tile([128, G, JE], F16)
            for w0 in range(0, G, 12):
                nc.sync.dma_start(out=wg_sb[:, w0:w0 + 12, :],
                                  in_=wg[:, w0:w0 + 12, :])
            wt07_sb = cpool.tile([128, G, 128], F16)
            for w0 in range(0, G, 18):
                nc.sync.dma_start(out=wt07_sb[:, w0:w0 + 18, :],
                                  in_=wt07[:, w0:w0 + 18, :])
            wt89_sb = cpool.tile([32, G, 128], F16)
            nc.sync.dma_start(out=wt89_sb[:], in_=wt89[:])
            sel_sb = cpool.tile([128, 8, 128], F16)
            nc.sync.dma_start(out=sel_sb[:],
                              in_=sel[:].rearrange("t p f -> p t f"))
            ident_sb = cpool.tile([128, 128], F32)
            nc.sync.dma_start(out=ident_sb[:], in_=ident[:])
            jm7_sb = cpool.tile([128, 8], F16)
            nc.sync.dma_start(out=jm7_sb[:], in_=jm7[:])
            jm8_sb = cpool.tile([32, 2], F16)
            nc.sync.dma_start(out=jm8_sb[:], in_=jm8[:])

            bstate = stpool.tile([128, GG, JB], F16)
            c_sb = stpool.tile([128, GG, JB], F16)

            evac_state = {"i": 0}

            def evac(dst, src):
                """psum f32 -> sbuf copy on a round-robin engine."""
                rr = CFG["evac_rr"]
                e = rr[evac_state["i"] % len(rr)]
                evac_state["i"] += 1
                if e == "a":
                    nc.scalar.copy(dst, src)
                elif e == "p":
                    nc.gpsimd.tensor_copy(dst, src)
                else:
                    nc.vector.tensor_copy(dst, src)

            def emit_v(sf7, sf8, k, prescale=False, sbj_in=None):
                """sf7 [128, 32*n7] f32 sbuf: s in (j,e)-rows layout; for
                k==0 (s1) n7==1 (columns are b directly); else n7==10 and
                the diagonal j-block of each 32-col group is live.
                Squash -> final DMA (k==2) or new block-diag v tiles."""
                s_bj = sbj_in
                if sbj_in is None:
                    s_bj = spool.tile([32, JE], F32, tag="sbj")
                if sbj_in is not None:
                    pass
                elif True:
                    for j in range(8):
                        trj = pA.tile([128, JB], F32, tag="a")
                        nc.tensor.transpose(trj[0:32, 0:128],
                                            sf7[:, 32 * j:32 * j + 32],
                                            ident_sb[:])
                        nc.vector.tensor_copy(
                            s_bj[:, 16 * j:16 * j + 16],
                            trj[0:32, 16 * j:16 * j + 16])
                    for jj in range(2):
                        trj = pA.tile([128, JB], F32, tag="a")
                        nc.tensor.transpose(trj[0:32, 0:32],
                                            sf8[:, 32 * jj:32 * jj + 32],
                                            ident_sb[0:32, 0:32])
                        nc.vector.tensor_copy(
                            s_bj[:, 128 + 16 * jj:144 + 16 * jj],
                            trj[0:32, 16 * jj:16 * jj + 16])
                sq = spool.tile([32, JE], F32, tag="sq")
                nc.vector.tensor_mul(sq[:], s_bj[:], s_bj[:])
                s2 = spool.tile([32, Nout], F32, tag="s2")
                nc.vector.tensor_reduce(
                    s2[:], sq[:].rearrange("p (j e) -> p j e", j=Nout,
                                           e=Dout),
                    axis=AX.X, op=AL.add)
                if prescale:
                    s2s = spool.tile([32, Nout], F32, tag="s2s")
                    nc.vector.tensor_scalar_mul(s2s[:], s2[:], 0.01)
                    s2 = s2s
                se = spool.tile([32, Nout], F32, tag="se")
                nc.vector.tensor_scalar_add(se[:], s2[:], EPS)
                h = spool.tile([32, Nout], F32, tag="h")
                nc.scalar.activation(h[:], se[:], AF.Ln)
                hm = spool.tile([32, Nout], F32, tag="hm")
                nc.vector.tensor_scalar_mul(hm[:], h[:], -0.5)
                r = spool.tile([32, Nout], F32, tag="r")
                nc.scalar.activation(r[:], hm[:], AF.Exp)
                t2 = spool.tile([32, Nout], F32, tag="t2")
                nc.vector.tensor_scalar_add(t2[:], s2[:], 1.0)
                rt2 = spool.tile([32, Nout], F32, tag="rt2")
                nc.vector.reciprocal(rt2[:], t2[:])
                w1 = spool.tile([32, Nout], F32, tag="w1")
                nc.vector.tensor_mul(w1[:], r[:], rt2[:])
                sc = spool.tile([32, Nout], F32, tag="sc")
                nc.vector.tensor_mul(sc[:], s2[:], w1[:])
                if prescale:
                    scp = spool.tile([32, Nout], F32, tag="scp")
                    nc.vector.tensor_scalar_mul(scp[:], sc[:], 0.1)
                    sc = scp
                v_bj = spool.tile([32, JE], F32, tag="vbj")
                nc.vector.tensor_mul(
                    v_bj[:].rearrange("p (j e) -> p j e", j=Nout, e=Dout),
                    s_bj[:].rearrange("p (j e) -> p j e", j=Nout, e=Dout),
                    sc[:].unsqueeze(2).broadcast_to([32, Nout, Dout]))
                if k == 2:
                    nc.sync.dma_start(out=vout[:], in_=v_bj[:])
                    return None, None
                vT7 = pA.tile([128, JB], F32, tag="a")
                nc.tensor.transpose(vT7[:, 0:32], v_bj[:, 0:128],
                                    ident_sb[0:32, 0:32])
                vT8 = pA.tile([128, JB], F32, tag="a")
                nc.tensor.transpose(vT8[0:32, 0:32], v_bj[:, 128:160],
                                    ident_sb[0:32, 0:32])
                vbd07 = bdpool.tile([128, 8, 32], F16, tag="bd7")
                nc.vector.tensor_mul(
                    vbd07[:],
                    vT7[:, 0:32].unsqueeze(1).broadcast_to([128, 8, 32]),
                    jm7_sb[:].unsqueeze(2).broadcast_to([128, 8, 32]))
                vbd89 = bdpool.tile([32, 2, 32], F16, tag="bd8")
                nc.vector.tensor_mul(
                    vbd89[:],
                    vT8[0:32, 0:32].unsqueeze(1).broadcast_to([32, 2, 32]),
                    jm8_sb[:].unsqueeze(2).broadcast_to([32, 2, 32]))
                return vbd07, vbd89

            def split_mul(out8, in0_view, in1_view):
                """gang-batched x-broadcast mul [128, 8, Nout, BL]:
                first (8-mul_pool) sub-chunks on DVE, rest on Pool."""
                nd = 8 - CFG["mul_pool"]
                if nd > 0:
                    nc.vector.tensor_mul(
                        out8[:, 0:nd], in0_view[:, 0:nd], in1_view[:, 0:nd])
                if nd < 8:
                    nc.gpsimd.tensor_mul(
                        out8[:, nd:8], in0_view[:, nd:8], in1_view[:, nd:8])

            def consume_mul(gg, k):
                """cxd = c*x (compact) for gang gg."""
                cxd = cxpool.tile([128, 8, JB], F16, tag="cxd")
                split_mul(
                    cxd[:].rearrange("p d (j b) -> p d j b", j=Nout, b=BL),
                    c_sb[:, gg, :].rearrange("p (j b) -> p j b", j=Nout,
                                             b=BL)
                    .unsqueeze(1).broadcast_to([128, 8, Nout, BL]),
                    xq_sb[:, gg, :, :].unsqueeze(2)
                    .broadcast_to([128, 8, Nout, BL]))
                return cxd

            def consume_mm7(gg, k, cxd, d0, d1):
                nd = 1 if CFG["no_d"] else 8
                for d in range(d0, min(d1, nd)):
                    nc.tensor.matmul(pS7[:], wq_sb[:, gg, d, 0:128],
                                     cxd[:, d, :],
                                     start=(gg == 0 and d == 0),
                                     stop=(gg == GG - 1 and d == nd - 1))

            def consume_mm8(gg, k, cxd):
                nd = 1 if CFG["no_d"] else 8
                for d in range(nd):
                    nc.tensor.matmul(pS8, wq_sb[:, gg, d, 128:160],
                                     cxd[:, d, 256:320],
                                     start=(gg == 0 and d == 0),
                                     stop=(gg == GG - 1 and d == nd - 1))

            wv8s = {}

            def a_pair(gg, cp, vbd07, vbd89):
                """Wv matmuls + evac for chunk pair cp of gang gg."""
                if cp == 0:
                    wv8 = xwpool.tile([128, 8, JB], F16, tag="wv8")
                    wv8s[gg] = wv8
                wv8 = wv8s[gg]
                pwv = pWv.tile([128, 2, 512], F32, tag="wv")
                if not CFG["no_a"]:
                    for h in (0, 1):
                        g = gg * 8 + 2 * cp + h
                        nc.tensor.matmul(
                            pwv[:, h, 0:256], wt07_sb[:, g, :],
                            vbd07[:].rearrange("p a b -> p (a b)"),
                            start=True, stop=True)
                    for h in (0, 1):
                        g = gg * 8 + 2 * cp + h
                        nc.tensor.matmul(
                            pwv[:, h, 256:320], wt89_sb[:, g, :],
                            vbd89[:].rearrange("p a b -> p (a b)"),
                            start=True, stop=True)
                elif gg == 0:
                    nc.vector.memset(pwv[:, :, 0:320], 0.5)
                if not CFG["no_evac"]:
                    evac(wv8[:, 2 * cp:2 * cp + 2, :], pwv[:, :, 0:320])
                elif gg == 0:
                    nc.vector.memset(wv8[:, 2 * cp:2 * cp + 2, :], 0.5)

            def a_mul(gg):
                """x-broadcast mul -> xwv(gg)."""
                wv8 = wv8s.pop(gg)
                xwv = xwpool.tile([128, 8, JB], F16, tag="xwv")
                if not CFG["no_mul"]:
                    split_mul(
                        xwv[:].rearrange("p c (j b) -> p c j b", j=Nout,
                                         b=BL),
                        wv8[:].rearrange("p c (j b) -> p c j b", j=Nout,
                                         b=BL),
                        xd_sb[:, gg * 8:gg * 8 + 8, :].unsqueeze(2)
                        .broadcast_to([128, 8, Nout, BL]))
                elif gg == 0:
                    nc.vector.memset(xwv[:], 0.5)
                return xwv

            def stage_c(gg, k, xwv):
                """d-reduce matmuls + b update + softmax -> c_sb[:, gg]."""
                pa = pA.tile([128, JB], F32, tag="a")
                if not CFG["no_c"]:
                    for c in range(8):
                        nc.tensor.matmul(pa[:], sel_sb[:, c, :],
                                         xwv[:, c, :],
                                         start=(c == 0), stop=(c == 7))
                else:
                    nc.tensor.matmul(pa[:], sel_sb[:, 0, :], xwv[:, 0, :],
                                     start=True, stop=True)
                expt = epool.tile([128, JB], F32, tag="exp")
                if CFG["no_soft"]:
                    evac(c_sb[:, gg, :], pa[:])
                    return
                if k == 1:
                    evac(bstate[:, gg, :], pa[:])
                    nc.scalar.activation(expt[:], pa[:], AF.Exp)
                else:
                    nc.vector.tensor_add(bstate[:, gg, :],
                                         bstate[:, gg, :], pa[:])
                    nc.scalar.activation(expt[:], bstate[:, gg, :], AF.Exp)
                Z = spool.tile([128, BL], F32, tag="Z")
                nc.vector.tensor_reduce(
                    Z[:], expt[:].rearrange("p (j b) -> p b j", j=Nout,
                                            b=BL),
                    axis=AX.X, op=AL.add)
                rz = spool.tile([128, BL], F32, tag="rz")
                nc.vector.reciprocal(rz[:], Z[:])
                nc.gpsimd.tensor_mul(
                    c_sb[:, gg, :].rearrange("p (j b) -> p j b", j=Nout,
                                             b=BL),
                    expt[:].rearrange("p (j b) -> p j b", j=Nout, b=BL),
                    rz[:].unsqueeze(1).broadcast_to([128, Nout, BL]))

            def routing_pass(k, vbd07, vbd89):
                xwvs = {}
                L = CFG["lag"]
                cxds = {}
                for gg in range(GG + L):
                    prev = gg - L
                    if 0 <= prev < GG:
                        cxds[prev] = consume_mul(prev, k)
                    for cp in range(4):
                        if gg < GG:
                            a_pair(gg, cp, vbd07, vbd89)
                        if 0 <= prev < GG:
                            consume_mm7(prev, k, cxds[prev],
                                        2 * cp, 2 * cp + 2)
                    if 0 <= prev < GG:
                        consume_mm8(prev, k, cxds.pop(prev))
                    if gg < GG:
                        xwvs[gg] = a_mul(gg)
                    if 0 <= gg - 1 < GG:
                        stage_c(gg - 1, k, xwvs.pop(gg - 1))
                # evacuate full s psum, extract diagonal via transposes
                sf7 = spool.tile([128, JB], F32, tag="sf7")
                nc.scalar.copy(sf7[:], pS7[:])
                sf8 = spool.tile([32, 64], F32, tag="sf8")
                nc.scalar.copy(sf8[:], pS8)
                return emit_v(sf7[:], sf8[:], k)

            def body():
                nonlocal pS7, pS8
                # s1: uniform coupling, straight from compact x
                ps1 = pA.tile([128, JB], F32, tag="a")
                for g in range(G):
                    nc.tensor.matmul(ps1[0:32, 0:160], xd_sb[:, g, :],
                                     wg_sb[:, g, :],
                                     start=(g == 0), stop=(g == G - 1))
                s1bj = spool.tile([32, JE], F32, tag="s1bj")
                nc.scalar.copy(s1bj[:], ps1[0:32, 0:160])
                vbd07, vbd89 = emit_v(None, None, 0, prescale=True,
                                      sbj_in=s1bj)
                for k in (1, 2):
                    t7 = pS.tile([128, JB], F32, tag="s7")
                    t8 = pS.tile([32, 64], F32, tag="s8")
                    pS7, pS8 = t7[:], t8[:]
                    vbd07, vbd89 = routing_pass(k, vbd07, vbd89)

            pS7 = pS8 = None
            if reps == 1 and inner == 1:
                body()
            elif reps < 0:   # unrolled (sim only)
                for _ in range(-reps):
                    body()
            else:
                with tc.For_i(0, reps, 1):
                    for _ in range(inner):
                        body()
    nc.compile()
    return nc


_NC = None


def _get_nc():
    global _NC
    if _NC is None:
        _NC = _build_program()
    return _NC


# ------------------------------------------------------------------ entry
def make_in_maps(x, W):
    x = np.asarray(x, dtype=np.float32)
    W = np.asarray(W, dtype=np.float32)
    wq_host = _prep_wq(W)
    wg_host = _prep_wg(W)
    wt07_host, wt89_host = _prep_wT(W)
    sel, ident, jm7, jm8 = _consts()
    in_maps = []
    for c in range(NCORES):
        xc = x[c * BL:(c + 1) * BL]
        in_maps.append({
            "xd": _prep_xd(xc),
            "xq": _prep_xq(xc),
            "wq": wq_host,
            "wg": wg_host,
            "wt07": wt07_host,
            "wt89": wt89_host,
            "sel": sel,
            "ident": ident,
            "jm7": jm7,
            "jm8": jm8,
        })
    return in_maps


def kernel(x, W):
    nc = _get_nc()
    in_maps = make_in_maps(x, W)
    res = run_bass_kernel_spmd(nc, in_maps, core_ids=list(range(NCORES)))
    out = np.concatenate([res.results[c]["vout"] for c in range(NCORES)],
                         axis=0)
    # device layout is j-major [B, (j e)] -> [B, j, e]
    return out.reshape(B, Nout, Dout).astype(np.float32)


if __name__ == "__main__":
    rng = np.random.default_rng(0)
    x = rng.standard_normal((B, Nin, Din)).astype(np.float32)
    W = (rng.standard_normal((Nin, Nout, Din, Dout)) * 0.35).astype(
        np.float32)
    v = kernel(x, W)
    print("out", v.shape, v.dtype, float(np.abs(v).max()))
